# revision 19
# baseline (speedup 1.0000x reference)
"""Trainium2 Bass kernel for nn_Net_91268055040039 (dense_mlp).

Computes out[b] = sum_{t,p} x[b,t,p] * |W[t,p]| * fc1_w[0, t*P+p] + fc1_b
  x: [32, 400, 10000] f32, W: [400, 10000] f32, fc1_w: [1, 4000000] f32.

Strategy (v5, fp16 + 3 compute lanes + tuned stream): shard T=400 into 8
slices of 50 rows. x/W/fc1 cast to fp16 on host (halves HBM traffic, the
binding resource; quantization rel-err ~4e-3 vs the 2e-2 gate). FREE padded
3907->3908 for DVE 2x-mode 4B alignment.

Stream plan (SWDGE ring, dense 2MB-max dram blocks, measured 389 GB/s):
  W(1MB) -> fc1(1MB) -> b0(1MB) -> 13x 2-batch chunks -> b27,b28,b29 single.
  ACT abs(W) overlaps the fc1 transfer; v=|W|*fc1 is ready ~when b0 lands.
  b30/b31 are prefetched CONCURRENTLY on the idle sync (HWDGE) ring and
  computed mid-stream, so the post-stream tail is one fused stt + out chain.

Compute lanes (measured: DVE TT pair 4.15us, stt 4.3us, ACT full reduce
3.83us, ACT 512-wide psum pick 1.13us, PE 8x512 matmul chain ~2-4us):
  - every batch: DVE tensor_tensor multiply (2x mode), mostly 2 batches per
    op against a replicated v to amortize op overhead
  - 18 batches: PE ones-matmul partition-reduce into a psum bank row
    (6 banks x 3 quad offsets {0,32,64}), then cheap ACT pick -> acc[q, b]
  - 11 batches: ACT full free-dim reduce -> acc[:, b]
  - b29 (last swdge batch) + b30/b31 (prefetched): fused DVE stt
acc is zeroed up front; every path leaves out[b] = colsum(acc[:, b]), so one
PE ones-matmul -> psum[1,32] -> copy -> DMA finishes the kernel.
Host sums the 8 per-core partials in f64 and adds fc1_b.
"""

import numpy as np

import concourse.bass as bass
import concourse.bacc as bacc
import concourse.mybir as mybir
from concourse.tile import TileContext
from concourse.bass_utils import run_bass_kernel_spmd

B, T, P = 32, 400, 10000
NCORES = 8
TS = T // NCORES          # 50 T-rows per core
K = TS * P                # 500000 reduction elements per core per batch
PART = 128
FREE = 3908               # ceil(K/128)=3907, padded to even for 4B alignment
KPAD = PART * FREE        # 500224 (224 zero pad)
# SWDGE x chunks cover b0..b29; b30/b31 ride the sync ring.
CHUNKS = [1] + [2] * 14 + [1]
ACT_BATCHES = frozenset(range(1, 22, 2))               # 11 full ACT reduces
PE_BATCHES = tuple(range(0, 29, 2)) + (23, 25, 27, 30, 31)   # 20 PE reduces
BANK = 512                # psum bank width in f32
NPB = 7                   # psum bank tiles for PE reduces (x3 quads = 21)
F16 = mybir.dt.float16
F32 = mybir.dt.float32

# Set by the test harness to capture an NTFF profile; harmless when False.
TRACE = False
LAST_RESULT = None


def build_program() -> bass.Bass:
    # Bacc (not raw Bass): its compile() splits multi-sem waits into separate
    # instructions - this neuronxcc build allows only 1 sync-wait per inst.
    nc = bacc.Bacc()
    xcs = [
        nc.declare_dram_parameter(f"xs{g}", [PART, nb * FREE], F16, isOutput=False)
        for g, nb in enumerate(CHUNKS)
    ]
    x3031d = nc.declare_dram_parameter(
        "x3031", [PART, 2 * FREE], F16, isOutput=False
    )
    # wf[:, :FREE] = W shard, wf[:, FREE:] = fc1 shard (one DMA, 15.6KB runs
    # per partition: measured faster than two 7.8KB-run transfers).
    wfd = nc.declare_dram_parameter("wf", [PART, 2 * FREE], F16, isOutput=False)
    out = nc.declare_dram_parameter("out", [1, B], F32, isOutput=True)

    # 8 accumulating matmul windows covering FREE=3908 into one 512-col bank.
    # Window 0 (start=True, full width) resets every psum col; the 324-wide
    # remainder accumulates onto cols 0:324.
    wins = [(w, BANK, w == 0, False) for w in range(0, 7 * BANK, BANK)]
    wins.append((7 * BANK, FREE - 7 * BANK, False, True))

    pe_slot = {b: i for i, b in enumerate(PE_BATCHES)}

    with TileContext(nc) as tc:
        with (
            tc.tile_pool(name="const", bufs=1) as cpool,
            tc.tile_pool(name="xp", bufs=7) as xpool,
            tc.tile_pool(name="sp", bufs=3) as spool,
            tc.tile_pool(name="psum", bufs=1, space="PSUM") as ppool,
        ):
            # Params first on the SWDGE ring; b30/b31 prefetch rides the
            # ACT-issued HWDGE ring concurrently.
            wft = cpool.tile([PART, 2 * FREE], F16)
            nc.gpsimd.dma_start(out=wft, in_=wfd[:, :])

            wabs = wft[:, :FREE]
            nc.scalar.activation(
                out=wabs, in_=wabs, func=mybir.ActivationFunctionType.Abs
            )
            # v2 = [v, v] so a paired TT can process 2 batches in one op.
            v2 = cpool.tile([PART, 2 * FREE], F16)
            v = v2[:, :FREE]
            nc.vector.tensor_tensor(
                out=v, in0=wabs, in1=wft[:, FREE:], op=mybir.AluOpType.mult
            )
            nc.scalar.copy(v2[:, FREE:], v)

            ones = cpool.tile([PART, 1], F32)
            nc.vector.memset(ones, 1.0)
            ones16 = cpool.tile([PART, 1], F16)
            nc.vector.memset(ones16, 1.0)
            acc = cpool.tile([PART, B], F32)
            nc.vector.memset(acc, 0.0)
            dump = cpool.tile([PART, FREE], F16)
            pbank = [
                ppool.tile([PART, BANK], F32, name=f"pbank{i}")
                for i in range(NPB)
            ]

            def reduce_batch(b, sc, off):
                """Free-dim reduce of sc[:, off:off+FREE] into acc[:, b]."""
                if b in pe_slot:
                    s = pe_slot[b]
                    pt = pbank[s % NPB]
                    q = 32 * (s // NPB)
                    for w0, nw, st, sp in wins:
                        nc.tensor.matmul(
                            out=pt[q : q + 1, :nw],
                            lhsT=ones16,
                            rhs=sc[:, off + w0 : off + w0 + nw],
                            start=st,
                            stop=sp,
                            skip_group_check=True,
                        )
                    nc.scalar.activation(
                        out=dump[q : q + 1, :BANK],
                        in_=pt[q : q + 1, :],
                        func=mybir.ActivationFunctionType.Copy,
                        accum_out=acc[q : q + 1, b : b + 1],
                    )
                else:
                    nc.scalar.activation(
                        out=dump,
                        in_=sc[:, off : off + FREE],
                        func=mybir.ActivationFunctionType.Copy,
                        accum_out=acc[:, b : b + 1],
                    )

            def stt_batch(b, xin):
                sc = spool.tile([PART, 2 * FREE], F16, tag="sc")
                nc.vector.scalar_tensor_tensor(
                    out=sc[:, :FREE],
                    in0=xin,
                    scalar=0.0,
                    in1=v,
                    op0=mybir.AluOpType.bypass,
                    op1=mybir.AluOpType.mult,
                    accum_out=acc[:, b : b + 1],
                )

            def tt_single(b, xsrc):
                sc = spool.tile([PART, 2 * FREE], F16, tag="sc")
                nc.vector.tensor_tensor(
                    out=sc[:, :FREE], in0=xsrc, in1=v, op=mybir.AluOpType.mult
                )
                reduce_batch(b, sc, 0)

            b0 = 0
            for g, nb in enumerate(CHUNKS):
                xt = xpool.tile([PART, 2 * FREE], F16, tag="xt")
                nc.gpsimd.dma_start(out=xt[:, : nb * FREE], in_=xcs[g][:, :])
                if b0 + nb - 1 == 29:
                    # Last swdge batch: fused stt keeps the tail short.
                    stt_batch(29, xt[:, :FREE])
                elif nb == 2:
                    # One paired TT computes both batches' products.
                    sc = spool.tile([PART, 2 * FREE], F16, tag="sc")
                    nc.vector.tensor_tensor(
                        out=sc, in0=xt, in1=v2, op=mybir.AluOpType.mult
                    )
                    reduce_batch(b0, sc, 0)
                    reduce_batch(b0 + 1, sc, FREE)
                else:
                    sc = spool.tile([PART, 2 * FREE], F16, tag="sc")
                    nc.vector.tensor_tensor(
                        out=sc[:, :FREE], in0=xt[:, :FREE], in1=v,
                        op=mybir.AluOpType.mult,
                    )
                    reduce_batch(b0, sc, 0)
                b0 += nb
                if g == 6:
                    # Prefetch b30/b31 on the sync (SP) HWDGE ring. The tile
                    # comes from the x pool (8th allocation -> buffer 0), so
                    # the DMA's buffer-free dependency delays it to ~t=21us:
                    # a dep-free DMA gets hoisted to t=0 by the scheduler and
                    # contends with the early swdge stream (-80 GB/s). SP
                    # blocking on the wait is harmless (it only does the
                    # final output DMA).
                    xt3031 = xpool.tile([PART, 2 * FREE], F16, tag="xt")
                    nc.sync.dma_start(out=xt3031, in_=x3031d[:, :])
                elif g == 8:
                    # Paired TT for the prefetched batches, PE-path reduces.
                    sc = spool.tile([PART, 2 * FREE], F16, tag="sc")
                    nc.vector.tensor_tensor(
                        out=sc, in0=xt3031, in1=v2, op=mybir.AluOpType.mult
                    )
                    reduce_batch(30, sc, 0)
                    reduce_batch(31, sc, FREE)

            ps = ppool.tile([1, B], F32)
            nc.tensor.matmul(out=ps, lhsT=ones, rhs=acc, start=True, stop=True)
            res = cpool.tile([1, B], F32)
            nc.scalar.copy(res, ps)
            nc.sync.dma_start(out=out[:, :], in_=res)
    nc.finalize()
    return nc


def _to_partition_major(flat: np.ndarray) -> np.ndarray:
    """[N, K] (f16) row-major -> [PART, N*FREE] where each partition's rows
    for consecutive N are adjacent."""
    n = flat.shape[0]
    padded = np.zeros((n, KPAD), dtype=np.float16)
    padded[:, :K] = flat
    # [n, PART, FREE] -> [PART, n, FREE] -> [PART, n*FREE]
    return np.ascontiguousarray(
        padded.reshape(n, PART, FREE).transpose(1, 0, 2)
    ).reshape(PART, n * FREE)


def make_in_maps(x: np.ndarray, W: np.ndarray, fc1_w: np.ndarray):
    x16 = np.asarray(x).astype(np.float16)
    W16 = np.asarray(W).astype(np.float16)
    f16 = np.asarray(fc1_w).astype(np.float16).reshape(T, P)
    in_maps = []
    for c in range(NCORES):
        t0 = c * TS
        xs = _to_partition_major(x16[:, t0 : t0 + TS, :].reshape(B, K))
        ws = _to_partition_major(W16[t0 : t0 + TS, :].reshape(1, K))
        fs = _to_partition_major(f16[t0 : t0 + TS, :].reshape(1, K))
        m = {
            "wf": np.concatenate([ws, fs], axis=1),
            "x3031": np.ascontiguousarray(xs[:, 30 * FREE : 32 * FREE]),
        }
        b0 = 0
        for g, nb in enumerate(CHUNKS):
            m[f"xs{g}"] = np.ascontiguousarray(
                xs[:, b0 * FREE : (b0 + nb) * FREE]
            )
            b0 += nb
        in_maps.append(m)
    return in_maps


def kernel(x, W, fc1_w, fc1_b):
    global LAST_RESULT
    nc = build_program()
    in_maps = make_in_maps(x, W, fc1_w)
    res = run_bass_kernel_spmd(
        nc, in_maps, core_ids=list(range(NCORES)), trace=TRACE
    )
    LAST_RESULT = res
    partial = np.zeros(B, dtype=np.float64)
    for r in res.results:
        partial += r["out"][0].astype(np.float64)
    out = partial.astype(np.float32) + np.float32(np.asarray(fc1_b).reshape(-1)[0])
    return out.reshape(B, 1).astype(np.float32)


# revision 20
# speedup vs baseline: 1.1538x; 1.1538x over previous
"""Trainium2 Bass kernel for nn_Net_91268055040039 (dense_mlp).

Computes out[b] = sum_{t,p} x[b,t,p] * |W[t,p]| * fc1_w[0, t*P+p] + fc1_b
  x: [32, 400, 10000] f32, W: [400, 10000] f32, fc1_w: [1, 4000000] f32.

Strategy (v5, fp16 + 3 compute lanes + tuned stream): shard T=400 into 8
slices of 50 rows. x/W/fc1 cast to fp16 on host (halves HBM traffic, the
binding resource; quantization rel-err ~4e-3 vs the 2e-2 gate). FREE padded
3907->3908 for DVE 2x-mode 4B alignment.

Stream plan (SWDGE ring, dense 2MB-max dram blocks, measured 389 GB/s):
  W(1MB) -> fc1(1MB) -> b0(1MB) -> 13x 2-batch chunks -> b27,b28,b29 single.
  ACT abs(W) overlaps the fc1 transfer; v=|W|*fc1 is ready ~when b0 lands.
  b30/b31 are prefetched CONCURRENTLY on the idle sync (HWDGE) ring and
  computed mid-stream, so the post-stream tail is one fused stt + out chain.

Compute lanes (measured: DVE TT pair 4.15us, stt 4.3us, ACT full reduce
3.83us, ACT 512-wide psum pick 1.13us, PE 8x512 matmul chain ~2-4us):
  - every batch: DVE tensor_tensor multiply (2x mode), mostly 2 batches per
    op against a replicated v to amortize op overhead
  - 18 batches: PE ones-matmul partition-reduce into a psum bank row
    (6 banks x 3 quad offsets {0,32,64}), then cheap ACT pick -> acc[q, b]
  - 11 batches: ACT full free-dim reduce -> acc[:, b]
  - b29 (last swdge batch) + b30/b31 (prefetched): fused DVE stt
acc is zeroed up front; every path leaves out[b] = colsum(acc[:, b]), so one
PE ones-matmul -> psum[1,32] -> copy -> DMA finishes the kernel.
Host sums the 8 per-core partials in f64 and adds fc1_b.
"""

import numpy as np

import concourse.bass as bass
import concourse.bacc as bacc
import concourse.mybir as mybir
from concourse.tile import TileContext
from concourse.bass_utils import run_bass_kernel_spmd

B, T, P = 32, 400, 10000
NCORES = 8
TS = T // NCORES          # 50 T-rows per core
K = TS * P                # 500000 reduction elements per core per batch
PART = 128
FREE = 3908               # ceil(K/128)=3907, padded to even for 4B alignment
KPAD = PART * FREE        # 500224 (224 zero pad)
# SWDGE x chunks cover b0..b29; b30/b31 ride the sync ring.
CHUNKS = [1] + [2] * 13 + [1, 1, 1]
ACT_BATCHES = frozenset(range(1, 22, 2))               # 11 full ACT reduces
PE_BATCHES = tuple(range(0, 29, 2)) + (23, 25, 27, 30, 31)   # 20 PE reduces
BANK = 512                # psum bank width in f32
NPB = 7                   # psum bank tiles for PE reduces (x3 quads = 21)
F16 = mybir.dt.float16
F32 = mybir.dt.float32

# Set by the test harness to capture an NTFF profile; harmless when False.
TRACE = False
LAST_RESULT = None


def build_program() -> bass.Bass:
    # Bacc (not raw Bass): its compile() splits multi-sem waits into separate
    # instructions - this neuronxcc build allows only 1 sync-wait per inst.
    nc = bacc.Bacc()
    xcs = [
        nc.declare_dram_parameter(f"xs{g}", [PART, nb * FREE], F16, isOutput=False)
        for g, nb in enumerate(CHUNKS)
    ]
    x3031d = nc.declare_dram_parameter(
        "x3031", [PART, 2 * FREE], F16, isOutput=False
    )
    # wf[:, :FREE] = W shard, wf[:, FREE:] = fc1 shard (one DMA, 15.6KB runs
    # per partition: measured faster than two 7.8KB-run transfers).
    wfd = nc.declare_dram_parameter("wf", [PART, 2 * FREE], F16, isOutput=False)
    out = nc.declare_dram_parameter("out", [1, B], F32, isOutput=True)

    # 8 accumulating matmul windows covering FREE=3908 into one 512-col bank.
    # Window 0 (start=True, full width) resets every psum col; the 324-wide
    # remainder accumulates onto cols 0:324.
    wins = [(w, BANK, w == 0, False) for w in range(0, 7 * BANK, BANK)]
    wins.append((7 * BANK, FREE - 7 * BANK, False, True))

    pe_slot = {b: i for i, b in enumerate(PE_BATCHES)}

    with TileContext(nc) as tc:
        with (
            tc.tile_pool(name="const", bufs=1) as cpool,
            tc.tile_pool(name="xp", bufs=6) as xpool,
            tc.tile_pool(name="sp", bufs=3) as spool,
            tc.tile_pool(name="psum", bufs=1, space="PSUM") as ppool,
        ):
            # Params first on the SWDGE ring; b30/b31 prefetch rides the
            # ACT-issued HWDGE ring concurrently.
            wft = cpool.tile([PART, 2 * FREE], F16)
            nc.gpsimd.dma_start(out=wft, in_=wfd[:, :])
            xt3031 = cpool.tile([PART, 2 * FREE], F16)

            wabs = wft[:, :FREE]
            nc.scalar.activation(
                out=wabs, in_=wabs, func=mybir.ActivationFunctionType.Abs
            )
            # v2 = [v, v] so a paired TT can process 2 batches in one op.
            v2 = cpool.tile([PART, 2 * FREE], F16)
            v = v2[:, :FREE]
            nc.vector.tensor_tensor(
                out=v, in0=wabs, in1=wft[:, FREE:], op=mybir.AluOpType.mult
            )
            nc.scalar.copy(v2[:, FREE:], v)

            ones = cpool.tile([PART, 1], F32)
            nc.vector.memset(ones, 1.0)
            ones16 = cpool.tile([PART, 1], F16)
            nc.vector.memset(ones16, 1.0)
            acc = cpool.tile([PART, B], F32)
            nc.vector.memset(acc, 0.0)
            dump = cpool.tile([PART, FREE], F16)
            pbank = [
                ppool.tile([PART, BANK], F32, name=f"pbank{i}")
                for i in range(NPB)
            ]

            def reduce_batch(b, sc, off):
                """Free-dim reduce of sc[:, off:off+FREE] into acc[:, b]."""
                if b in pe_slot:
                    s = pe_slot[b]
                    pt = pbank[s % NPB]
                    q = 32 * (s // NPB)
                    for w0, nw, st, sp in wins:
                        nc.tensor.matmul(
                            out=pt[q : q + 1, :nw],
                            lhsT=ones16,
                            rhs=sc[:, off + w0 : off + w0 + nw],
                            start=st,
                            stop=sp,
                            skip_group_check=True,
                        )
                    nc.scalar.activation(
                        out=dump[q : q + 1, :BANK],
                        in_=pt[q : q + 1, :],
                        func=mybir.ActivationFunctionType.Copy,
                        accum_out=acc[q : q + 1, b : b + 1],
                    )
                else:
                    nc.scalar.activation(
                        out=dump,
                        in_=sc[:, off : off + FREE],
                        func=mybir.ActivationFunctionType.Copy,
                        accum_out=acc[:, b : b + 1],
                    )

            def stt_batch(b, xin):
                sc = spool.tile([PART, 2 * FREE], F16, tag="sc")
                nc.vector.scalar_tensor_tensor(
                    out=sc[:, :FREE],
                    in0=xin,
                    scalar=0.0,
                    in1=v,
                    op0=mybir.AluOpType.bypass,
                    op1=mybir.AluOpType.mult,
                    accum_out=acc[:, b : b + 1],
                )

            def tt_single(b, xsrc):
                sc = spool.tile([PART, 2 * FREE], F16, tag="sc")
                nc.vector.tensor_tensor(
                    out=sc[:, :FREE], in0=xsrc, in1=v, op=mybir.AluOpType.mult
                )
                reduce_batch(b, sc, 0)

            b0 = 0
            for g, nb in enumerate(CHUNKS):
                xt = xpool.tile([PART, 2 * FREE], F16, tag="xt")
                nc.gpsimd.dma_start(out=xt[:, : nb * FREE], in_=xcs[g][:, :])
                if b0 + nb - 1 == 29:
                    # Last swdge batch: fused stt keeps the tail short.
                    stt_batch(29, xt[:, :FREE])
                elif nb == 2:
                    # One paired TT computes both batches' products.
                    sc = spool.tile([PART, 2 * FREE], F16, tag="sc")
                    nc.vector.tensor_tensor(
                        out=sc, in0=xt, in1=v2, op=mybir.AluOpType.mult
                    )
                    reduce_batch(b0, sc, 0)
                    reduce_batch(b0 + 1, sc, FREE)
                else:
                    sc = spool.tile([PART, 2 * FREE], F16, tag="sc")
                    nc.vector.tensor_tensor(
                        out=sc[:, :FREE], in0=xt[:, :FREE], in1=v,
                        op=mybir.AluOpType.mult,
                    )
                    reduce_batch(b0, sc, 0)
                b0 += nb
                if g == 4:
                    # Prefetch b30/b31 on the ACT-issued HWDGE ring (one 2MB
                    # transfer; the scheduler hoists it early regardless).
                    nc.scalar.dma_start(out=xt3031, in_=x3031d[:, :])
                elif g == 8:
                    # Paired TT for the prefetched batches, PE-path reduces.
                    sc = spool.tile([PART, 2 * FREE], F16, tag="sc")
                    nc.vector.tensor_tensor(
                        out=sc, in0=xt3031, in1=v2, op=mybir.AluOpType.mult
                    )
                    reduce_batch(30, sc, 0)
                    reduce_batch(31, sc, FREE)

            ps = ppool.tile([1, B], F32)
            nc.tensor.matmul(out=ps, lhsT=ones, rhs=acc, start=True, stop=True)
            res = cpool.tile([1, B], F32)
            nc.scalar.copy(res, ps)
            nc.sync.dma_start(out=out[:, :], in_=res)
    nc.finalize()
    return nc


def _to_partition_major(flat: np.ndarray) -> np.ndarray:
    """[N, K] (f16) row-major -> [PART, N*FREE] where each partition's rows
    for consecutive N are adjacent."""
    n = flat.shape[0]
    padded = np.zeros((n, KPAD), dtype=np.float16)
    padded[:, :K] = flat
    # [n, PART, FREE] -> [PART, n, FREE] -> [PART, n*FREE]
    return np.ascontiguousarray(
        padded.reshape(n, PART, FREE).transpose(1, 0, 2)
    ).reshape(PART, n * FREE)


def make_in_maps(x: np.ndarray, W: np.ndarray, fc1_w: np.ndarray):
    x16 = np.asarray(x).astype(np.float16)
    W16 = np.asarray(W).astype(np.float16)
    f16 = np.asarray(fc1_w).astype(np.float16).reshape(T, P)
    in_maps = []
    for c in range(NCORES):
        t0 = c * TS
        xs = _to_partition_major(x16[:, t0 : t0 + TS, :].reshape(B, K))
        ws = _to_partition_major(W16[t0 : t0 + TS, :].reshape(1, K))
        fs = _to_partition_major(f16[t0 : t0 + TS, :].reshape(1, K))
        m = {
            "wf": np.concatenate([ws, fs], axis=1),
            "x3031": np.ascontiguousarray(xs[:, 30 * FREE : 32 * FREE]),
        }
        b0 = 0
        for g, nb in enumerate(CHUNKS):
            m[f"xs{g}"] = np.ascontiguousarray(
                xs[:, b0 * FREE : (b0 + nb) * FREE]
            )
            b0 += nb
        in_maps.append(m)
    return in_maps


def kernel(x, W, fc1_w, fc1_b):
    global LAST_RESULT
    nc = build_program()
    in_maps = make_in_maps(x, W, fc1_w)
    res = run_bass_kernel_spmd(
        nc, in_maps, core_ids=list(range(NCORES)), trace=TRACE
    )
    LAST_RESULT = res
    partial = np.zeros(B, dtype=np.float64)
    for r in res.results:
        partial += r["out"][0].astype(np.float64)
    out = partial.astype(np.float32) + np.float32(np.asarray(fc1_b).reshape(-1)[0])
    return out.reshape(B, 1).astype(np.float32)
